# revision 1
# baseline (speedup 1.0000x reference)
"""Trainium2 Bass kernel for the LstmRnn problem (B=8192, T=48, F=64, H=128, OUT=24).

Strategy (pure data parallelism over 8 NeuronCores, 1024 batch rows each):
  * Everything on-device lives transposed as [feature, batch] so the hidden
    dim (128) sits on SBUF partitions and batch streams along the free dim.
  * Batch is split into 2 half-tiles of 512 columns that pipeline through
    the engines (PE -> ACT -> DVE/GPSIMD) across the sequential scan.
  * Gates are reordered to (i, f, o, g) so one Sigmoid instruction covers
    i,f,o contiguously in PSUM and one Tanh covers g.
  * The whole input sequence is SBUF-resident, packed [128, T/2, B] (even
    timesteps on partitions 0-63, odd on 64-127), prefetched in chunks at
    start. This removes all per-step input DMAs (HWDGE descriptors only
    support a single sync wait, so streaming tiles can't carry the deps).
  * Warmup biases come from K=1 matmuls (bias row x ones row), which double
    as the PSUM-slot WAR absorbers; decode biases ride a ones-row appended
    to pred: [pred;1] @ [W2;b2] (the output dense is rank-64, so the decode
    input matmul factors through pred).
  * Matmuls run in float32r (1 col/cycle on TRN2 vs 4 for plain fp32).
"""

import os
import sys

import numpy as np

for _p in ("/opt/trn_rl_repo",):
    if os.path.isdir(_p) and _p not in sys.path:
        sys.path.insert(0, _p)

import concourse.bacc as bacc
import concourse.bass as bass
import concourse.mybir as mybir
import concourse.tile as tile
from concourse.bass_utils import run_bass_kernel_spmd

B, T, F, H, OUT = 8192, 48, 64, 128, 24
NCORES = 8
BC = B // NCORES   # 1024 batch rows per core
HALF = BC // 2     # 512-wide half tiles
G4 = 4 * H
TP = T // 2        # timestep pairs in the packed layout

FP32 = mybir.dt.float32
FP32R = mybir.dt.float32r
AF = mybir.ActivationFunctionType
ALU = mybir.AluOpType

LAST_RESULT = None  # BassKernelResults of the most recent kernel() call


def build_nc():
    nc = bacc.Bacc("TRN2", target_bir_lowering=False, debug=False, enable_asserts=False)

    x_d = nc.declare_dram_parameter("x", [H, TP, BC], FP32R, isOutput=False)
    w1_d = nc.declare_dram_parameter("w1dup", [H, G4], FP32R, isOutput=False)
    b1_d = nc.declare_dram_parameter("b1row", [1, G4], FP32R, isOutput=False)
    u1_d = nc.declare_dram_parameter("u1", [H, G4], FP32R, isOutput=False)
    w2_d = nc.declare_dram_parameter("w2aug", [F + 1, G4], FP32R, isOutput=False)
    u2_d = nc.declare_dram_parameter("u2", [H, G4], FP32R, isOutput=False)
    wd1_d = nc.declare_dram_parameter("wd1", [H, H], FP32R, isOutput=False)
    wd_d = nc.declare_dram_parameter("wd", [H, H], FP32R, isOutput=False)
    bd1_d = nc.declare_dram_parameter("bd1", [H, 1], FP32, isOutput=False)
    bd_d = nc.declare_dram_parameter("bd", [F, 1], FP32, isOutput=False)
    ones_d = nc.declare_dram_parameter("onesrow", [1, HALF], FP32R, isOutput=False)
    zeros_d = nc.declare_dram_parameter("zeros", [H, HALF], FP32R, isOutput=False)
    out_d = nc.declare_dram_parameter("out", [OUT, F, BC], FP32R, isOutput=True)

    with tile.TileContext(nc) as tc:
        with (
            tc.tile_pool(name="wpool", bufs=1) as wp,
            tc.tile_pool(name="state", bufs=1) as sp,
            tc.tile_pool(name="psA", bufs=1, space="PSUM") as ppA,
            tc.tile_pool(name="psB", bufs=1, space="PSUM") as ppB,
        ):
            # ---- weights (resident) ----
            w1 = wp.tile([H, G4], FP32R, tag="w1", name="w1")
            b1r = wp.tile([1, G4], FP32R, tag="b1r", name="b1r")
            u1 = wp.tile([H, G4], FP32R, tag="u1", name="u1")
            w2 = wp.tile([F + 1, G4], FP32R, tag="w2", name="w2")
            u2 = wp.tile([H, G4], FP32R, tag="u2", name="u2")
            wd1 = wp.tile([H, H], FP32R, tag="wd1", name="wd1")
            wd = wp.tile([H, H], FP32R, tag="wd", name="wd")
            bd1 = wp.tile([H, 1], FP32, tag="bd1", name="bd1")
            bd = wp.tile([F, 1], FP32, tag="bd", name="bd")
            ones = wp.tile([1, HALF], FP32R, tag="ones", name="ones")
            for t_, d_ in ((w1, w1_d), (b1r, b1_d), (u1, u1_d), (w2, w2_d),
                           (u2, u2_d), (wd1, wd1_d), (wd, wd_d), (bd1, bd1_d),
                           (bd, bd_d)):
                nc.sync.dma_start(t_[:], d_[:])
            nc.sync.dma_start(ones[:], ones_d[:])

            # ---- whole input sequence, SBUF resident ----
            xsb = sp.tile([H, TP, BC], FP32R, tag="xsb", name="xsb")
            XCH = 4  # t-pairs per prefetch chunk
            for c in range(0, TP, XCH):
                hi = min(c + XCH, TP)
                nc.sync.dma_start(xsb[:, c:hi, :], x_d[:, c:hi, :])

            # 1x1 "observer" matmuls: advance the PE engine clock past every
            # weight-DMA lane tick and the ones-memset, so steady-state
            # matmuls never mix a DMA-sem wait with an engine-sem wait
            # (HW-decoded PE instructions can't carry that combination).
            for hf, pool in ((0, ppA), (1, ppB)):
                initz = pool.tile([H, 4, HALF], FP32, tag=f"z{hf}", name=f"initz{hf}")
                for src in (bd, b1r, u1, w2, u2, wd1, wd, bd1, ones):
                    s_ = src[0:1, 0:1].bitcast(FP32)
                    nc.tensor.matmul(
                        initz[0:1, 0, 0:1], s_, s_,
                        start=True, stop=True, skip_group_check=True,
                    )

            # ---- per-half persistent state ----
            halves = []
            for hf, pool in ((0, ppA), (1, ppB)):
                st = {
                    "h": sp.tile([H, HALF], FP32R, tag=f"h{hf}", name=f"h{hf}"),
                    "c": sp.tile([H, HALF], FP32, tag=f"c{hf}", name=f"c{hf}"),
                    "sifo": sp.tile([H, 3, HALF], FP32, tag=f"sifo{hf}", name=f"sifo{hf}"),
                    "tg": sp.tile([H, HALF], FP32, tag=f"tg{hf}", name=f"tg{hf}"),
                    "tc": sp.tile([H, HALF], FP32, tag=f"tc{hf}", name=f"tc{hf}"),
                    "m1": sp.tile([H, HALF], FP32, tag=f"m1{hf}", name=f"m1{hf}"),
                    "m2": sp.tile([H, HALF], FP32, tag=f"m2{hf}", name=f"m2{hf}"),
                    "x1": sp.tile([H, HALF], FP32R, tag=f"x1{hf}", name=f"x1{hf}"),
                    "x2": sp.tile([H, HALF], FP32R, tag=f"x2{hf}", name=f"x2{hf}"),
                    "pred": sp.tile([F + 1, HALF], FP32R, tag=f"pred{hf}", name=f"pred{hf}"),
                    "pool": pool,
                    "off": hf * HALF,
                    "tag": f"z{hf}",
                }
                halves.append(st)
                nc.sync.dma_start(st["h"][:], zeros_d[:])
                nc.vector.memset(st["c"][:], 0.0)
                nc.sync.dma_start(st["pred"][F : F + 1, :], ones_d[:])

            def elementwise(st, z):
                nc.scalar.activation(st["sifo"][:], z[:, 0:3, :], AF.Sigmoid)
                nc.scalar.activation(st["tg"][:], z[:, 3, :], AF.Tanh)
                nc.gpsimd.tensor_mul(st["m2"][:], st["sifo"][:, 0, :], st["tg"][:])
                nc.vector.tensor_mul(st["m1"][:], st["sifo"][:, 1, :], st["c"][:])
                nc.vector.tensor_add(st["c"][:], st["m1"][:], st["m2"][:])
                nc.scalar.activation(st["tc"][:], st["c"][:], AF.Tanh)
                nc.gpsimd.tensor_mul(st["h"][:], st["sifo"][:, 2, :], st["tc"][:])

            def warm_step(st, t):
                # z = b1 + x_t @ W1 + h @ U1, gates (i,f,o,g) in 4 PSUM banks
                z = st["pool"].tile([H, 4, HALF], FP32, tag=st["tag"], name="z" + st["tag"])
                par, j = t % 2, t // 2
                xa = xsb[64 * par : 64 * par + 64, j, st["off"] : st["off"] + HALF]
                wa = w1[64 * par : 64 * par + 64, :]
                for g in range(4):
                    # K=1 bias matmul; the g==0 one also absorbs the PSUM-slot
                    # WAR wait (HW-decoded PE instrs have only 2 wait slots).
                    nc.tensor.matmul(
                        z[:, g, :], b1r[0:1, g * H : (g + 1) * H], ones[:],
                        start=True, stop=False,
                    )
                for g in range(4):
                    nc.tensor.matmul(
                        z[:, g, :], wa[:, g * H : (g + 1) * H], xa,
                        start=False, stop=(t == 0),
                    )
                if t > 0:
                    for g in range(4):
                        nc.tensor.matmul(
                            z[:, g, :], u1[:, g * H : (g + 1) * H], st["h"][:],
                            start=False, stop=True,
                        )
                elementwise(st, z)

            def dec_step(st):
                # z = [pred;1] @ [W2;b2] + h @ U2
                z = st["pool"].tile([H, 4, HALF], FP32, tag=st["tag"], name="z" + st["tag"])
                for g in range(4):
                    nc.tensor.matmul(
                        z[:, g, :], w2[:, g * H : (g + 1) * H], st["pred"][:],
                        start=True, stop=False,
                    )
                for g in range(4):
                    nc.tensor.matmul(
                        z[:, g, :], u2[:, g * H : (g + 1) * H], st["h"][:],
                        start=False, stop=True,
                    )
                elementwise(st, z)

            def head(st, k):
                hd = st["pool"].tile([H, 3, HALF], FP32, tag=st["tag"], name="hd" + st["tag"])
                # 1x1 matmul absorbing the PSUM-slot WAR wait so the x1 matmul
                # carries only its RAW dependency.
                wdm = w1[0:1, 0:1].bitcast(FP32)
                nc.tensor.matmul(
                    hd[0:1, 0, 0:1], wdm, wdm,
                    start=True, stop=True, skip_group_check=True,
                )
                nc.tensor.matmul(hd[:, 0, :], wd1[:], st["h"][:])
                nc.vector.tensor_scalar(
                    st["x1"][:], hd[:, 0, :], bd1[:, 0:1], 0.0, ALU.add, ALU.max
                )
                nc.tensor.matmul(hd[:, 1, :], wd1[:], st["x1"][:])
                nc.vector.tensor_scalar(
                    st["x2"][:], hd[:, 1, :], bd1[:, 0:1], 0.0, ALU.add, ALU.max
                )
                nc.tensor.matmul(hd[:, 2, :], wd[:], st["x2"][:])
                nc.vector.tensor_scalar(
                    st["pred"][0:F, :], hd[0:F, 2, :], bd[:, 0:1], None, ALU.add
                )
                nc.sync.dma_start(
                    out_d[k, :, st["off"] : st["off"] + HALF], st["pred"][0:F, :]
                )

            # ---- warmup scan over the input sequence ----
            for t in range(T):
                for st in halves:
                    warm_step(st, t)

            # ---- autoregressive decode ----
            for st in halves:
                head(st, 0)
            for k in range(1, OUT):
                for st in halves:
                    dec_step(st)
                for st in halves:
                    head(st, k)

    nc.compile()
    return nc


_NC_CACHE = None


def _get_nc():
    global _NC_CACHE
    if _NC_CACHE is None:
        _NC_CACHE = build_nc()
    return _NC_CACHE


def _prep_weights(W1, U1, b1, W2, U2, b2, Wd1, bd1, Wd, bd):
    f32 = np.float32
    perm = np.concatenate(
        [np.arange(0, 128), np.arange(128, 256), np.arange(384, 512), np.arange(256, 384)]
    )
    W1p, U1p, b1p = W1[:, perm], U1[:, perm], b1[perm]
    W2p, U2p, b2p = W2[:, perm], U2[:, perm], b2[perm]
    w1dup = np.ascontiguousarray(np.concatenate([W1p, W1p], axis=0), f32)
    w2aug = np.ascontiguousarray(np.concatenate([W2p, b2p[None, :]], axis=0), f32)
    return {
        "w1dup": w1dup,
        "b1row": np.ascontiguousarray(b1p[None, :], f32),
        "u1": np.ascontiguousarray(U1p, f32),
        "w2aug": w2aug,
        "u2": np.ascontiguousarray(U2p, f32),
        "wd1": np.ascontiguousarray(Wd1, f32),
        "wd": np.ascontiguousarray(np.concatenate([Wd, np.zeros((H, H - F), np.float32)], axis=1), f32),
        "bd1": np.ascontiguousarray(bd1[:, None], f32),
        "bd": np.ascontiguousarray(bd[:, None], f32),
        "onesrow": np.ones((1, HALF), f32),
        "zeros": np.zeros((H, HALF), f32),
    }


def _prep_x(inputs):
    # inputs [Bn, T, F] -> [2F=128, T/2, Bn]: even timesteps on rows 0-63,
    # odd timesteps on rows 64-127
    xT = np.transpose(inputs, (1, 2, 0))           # [T, F, Bn]
    xp = np.concatenate([xT[0::2], xT[1::2]], axis=1)  # [T/2, 2F, Bn]
    return np.ascontiguousarray(np.transpose(xp, (1, 0, 2)), np.float32)


def _preprocess_single(inputs, W1, U1, b1, W2, U2, b2, Wd1, bd1, Wd, bd):
    m = _prep_weights(W1, U1, b1, W2, U2, b2, Wd1, bd1, Wd, bd)
    m["x"] = _prep_x(inputs)
    return m


def _preprocess(inputs, W1, U1, b1, W2, U2, b2, Wd1, bd1, Wd, bd):
    shared = _prep_weights(W1, U1, b1, W2, U2, b2, Wd1, bd1, Wd, bd)
    xpk = _prep_x(inputs)  # [128, T/2, B]
    in_maps = []
    for i in range(NCORES):
        m = dict(shared)
        m["x"] = np.ascontiguousarray(xpk[:, :, i * BC : (i + 1) * BC])
        in_maps.append(m)
    return in_maps


def kernel(**inputs):
    global LAST_RESULT
    args = {k: np.asarray(v) for k, v in inputs.items()}
    in_maps = _preprocess(**args)
    nc = _get_nc()
    res = run_bass_kernel_spmd(nc, in_maps, list(range(NCORES)))
    LAST_RESULT = res
    outs = [res.results[i]["out"] for i in range(NCORES)]  # each [OUT, F, BC]
    full = np.concatenate(outs, axis=2)  # [OUT, F, B]
    return np.ascontiguousarray(np.transpose(full, (2, 0, 1)), np.float32)



# revision 2
# speedup vs baseline: 2.3564x; 2.3564x over previous
"""Trainium2 Bass kernel for the LstmRnn problem (B=8192, T=48, F=64, H=128, OUT=24).

Strategy (pure data parallelism over 8 NeuronCores, 1024 batch rows each):
  * The end-to-end metric is dominated by host<->device transfer over the
    axon tunnel (~55 MB/s), so everything shipped is float16: the packed
    input sequence, all matmul weights, and the output. Matmuls run
    fp16 x fp16 with fp32 PSUM accumulation (also 4 cols/cycle on the PE
    vs 1 for fp32r); cell state c and all elementwise math stay fp32.
  * Everything on-device lives transposed as [feature, batch] so the hidden
    dim (128) sits on SBUF partitions and batch streams along the free dim.
  * Batch is split into 2 half-tiles of 512 columns that pipeline through
    the engines (PE -> ACT -> DVE/GPSIMD) across the sequential scan.
  * Gates are reordered to (i, f, o, g) so one Sigmoid instruction covers
    i,f,o contiguously in PSUM and one Tanh covers g.
  * The whole input sequence is SBUF-resident, packed [128, T/2, B] (even
    timesteps on partitions 0-63, odd on 64-127), prefetched in chunks at
    start. This removes all per-step input DMAs (HWDGE descriptors only
    support a single sync wait, so streaming tiles can't carry the deps).
  * Warmup biases come from K=1 matmuls (bias row x ones row), which double
    as the PSUM-slot WAR absorbers; decode biases ride a ones-row appended
    to pred: [pred;1] @ [W2;b2] (the output dense is rank-64, so the decode
    input matmul factors through pred).
"""

import os
import sys

import numpy as np

for _p in ("/opt/trn_rl_repo",):
    if os.path.isdir(_p) and _p not in sys.path:
        sys.path.insert(0, _p)

import concourse.bacc as bacc
import concourse.bass as bass
import concourse.mybir as mybir
import concourse.tile as tile
from concourse.bass_utils import run_bass_kernel_spmd

B, T, F, H, OUT = 8192, 48, 64, 128, 24
NCORES = 8
BC = B // NCORES   # 1024 batch rows per core
HALF = BC // 2     # 512-wide half tiles
G4 = 4 * H
TP = T // 2        # timestep pairs in the packed layout

FP32 = mybir.dt.float32
FP16 = mybir.dt.float16
AF = mybir.ActivationFunctionType
ALU = mybir.AluOpType

LAST_RESULT = None  # BassKernelResults of the most recent kernel() call


def build_nc():
    nc = bacc.Bacc("TRN2", target_bir_lowering=False, debug=False, enable_asserts=False)

    x_d = nc.declare_dram_parameter("x", [H, TP, BC], FP16, isOutput=False)
    w1_d = nc.declare_dram_parameter("w1dup", [H, G4], FP16, isOutput=False)
    b1_d = nc.declare_dram_parameter("b1row", [1, G4], FP16, isOutput=False)
    u1_d = nc.declare_dram_parameter("u1", [H, G4], FP16, isOutput=False)
    w2_d = nc.declare_dram_parameter("w2aug", [F + 1, G4], FP16, isOutput=False)
    u2_d = nc.declare_dram_parameter("u2", [H, G4], FP16, isOutput=False)
    wd1_d = nc.declare_dram_parameter("wd1", [H, H], FP16, isOutput=False)
    wd_d = nc.declare_dram_parameter("wd", [H, H], FP16, isOutput=False)
    bd1_d = nc.declare_dram_parameter("bd1", [H, 1], FP32, isOutput=False)
    bd_d = nc.declare_dram_parameter("bd", [F, 1], FP32, isOutput=False)
    ones_d = nc.declare_dram_parameter("onesrow", [1, HALF], FP16, isOutput=False)
    out_d = nc.declare_dram_parameter("out", [OUT, F, BC], FP16, isOutput=True)

    with tile.TileContext(nc) as tc:
        with (
            tc.tile_pool(name="wpool", bufs=1) as wp,
            tc.tile_pool(name="state", bufs=1) as sp,
            tc.tile_pool(name="psA", bufs=1, space="PSUM") as ppA,
            tc.tile_pool(name="psB", bufs=1, space="PSUM") as ppB,
        ):
            # ---- weights (resident) ----
            w1 = wp.tile([H, G4], FP16, tag="w1", name="w1")
            b1r = wp.tile([1, G4], FP16, tag="b1r", name="b1r")
            u1 = wp.tile([H, G4], FP16, tag="u1", name="u1")
            w2 = wp.tile([F + 1, G4], FP16, tag="w2", name="w2")
            u2 = wp.tile([H, G4], FP16, tag="u2", name="u2")
            wd1 = wp.tile([H, H], FP16, tag="wd1", name="wd1")
            wd = wp.tile([H, H], FP16, tag="wd", name="wd")
            bd1 = wp.tile([H, 1], FP32, tag="bd1", name="bd1")
            bd = wp.tile([F, 1], FP32, tag="bd", name="bd")
            ones = wp.tile([1, HALF], FP16, tag="ones", name="ones")
            for t_, d_ in ((w1, w1_d), (b1r, b1_d), (u1, u1_d), (w2, w2_d),
                           (u2, u2_d), (wd1, wd1_d), (wd, wd_d), (bd1, bd1_d),
                           (bd, bd_d)):
                nc.sync.dma_start(t_[:], d_[:])
            nc.sync.dma_start(ones[:], ones_d[:])

            # ---- whole input sequence, SBUF resident ----
            xsb = sp.tile([H, TP, BC], FP16, tag="xsb", name="xsb")
            XCH = 4  # t-pairs per prefetch chunk
            for c in range(0, TP, XCH):
                hi = min(c + XCH, TP)
                nc.sync.dma_start(xsb[:, c:hi, :], x_d[:, c:hi, :])

            # 1x1 "observer" matmuls: advance the PE engine clock past every
            # weight-DMA lane tick, so steady-state matmuls never mix a
            # DMA-sem wait with an engine-sem wait (HW-decoded PE
            # instructions can't carry that combination).
            for hf, pool in ((0, ppA), (1, ppB)):
                initz = pool.tile([H, 4, HALF], FP32, tag=f"z{hf}", name=f"initz{hf}")
                for src in (b1r, u1, w2, u2, wd1, wd, ones):
                    s_ = src[0:1, 0:1]
                    nc.tensor.matmul(
                        initz[0:1, 0, 0:1], s_, s_,
                        start=True, stop=True, skip_group_check=True,
                    )
                for src in (bd, bd1):
                    s_ = src[0:1, 0:1]
                    nc.tensor.matmul(
                        initz[0:1, 0, 0:1], s_, s_,
                        start=True, stop=True, skip_group_check=True,
                    )

            # ---- per-half persistent state ----
            halves = []
            for hf, pool in ((0, ppA), (1, ppB)):
                st = {
                    "h": sp.tile([H, HALF], FP16, tag=f"h{hf}", name=f"h{hf}"),
                    "c": sp.tile([H, HALF], FP32, tag=f"c{hf}", name=f"c{hf}"),
                    "sifo": sp.tile([H, 3, HALF], FP32, tag=f"sifo{hf}", name=f"sifo{hf}"),
                    "tg": sp.tile([H, HALF], FP32, tag=f"tg{hf}", name=f"tg{hf}"),
                    "tc": sp.tile([H, HALF], FP32, tag=f"tc{hf}", name=f"tc{hf}"),
                    "m1": sp.tile([H, HALF], FP32, tag=f"m1{hf}", name=f"m1{hf}"),
                    "m2": sp.tile([H, HALF], FP32, tag=f"m2{hf}", name=f"m2{hf}"),
                    "x1": sp.tile([H, HALF], FP16, tag=f"x1{hf}", name=f"x1{hf}"),
                    "x2": sp.tile([H, HALF], FP16, tag=f"x2{hf}", name=f"x2{hf}"),
                    "pred": sp.tile([F + 1, HALF], FP16, tag=f"pred{hf}", name=f"pred{hf}"),
                    "pool": pool,
                    "off": hf * HALF,
                    "tag": f"z{hf}",
                }
                halves.append(st)
                nc.vector.memset(st["c"][:], 0.0)
                nc.sync.dma_start(st["pred"][F : F + 1, :], ones_d[:])

            def elementwise(st, z):
                nc.scalar.activation(st["sifo"][:], z[:, 0:3, :], AF.Sigmoid)
                nc.scalar.activation(st["tg"][:], z[:, 3, :], AF.Tanh)
                nc.gpsimd.tensor_mul(st["m2"][:], st["sifo"][:, 0, :], st["tg"][:])
                nc.vector.tensor_mul(st["m1"][:], st["sifo"][:, 1, :], st["c"][:])
                nc.vector.tensor_add(st["c"][:], st["m1"][:], st["m2"][:])
                nc.scalar.activation(st["tc"][:], st["c"][:], AF.Tanh)
                nc.gpsimd.tensor_mul(st["h"][:], st["sifo"][:, 2, :], st["tc"][:])

            def warm_step(st, t):
                # z = b1 + x_t @ W1 + h @ U1, gates (i,f,o,g) in 4 PSUM banks
                z = st["pool"].tile([H, 4, HALF], FP32, tag=st["tag"], name="z" + st["tag"])
                par, j = t % 2, t // 2
                xa = xsb[64 * par : 64 * par + 64, j, st["off"] : st["off"] + HALF]
                wa = w1[64 * par : 64 * par + 64, :]
                for g in range(4):
                    # K=1 bias matmul; the g==0 one also absorbs the PSUM-slot
                    # WAR wait (HW-decoded PE instrs have only 2 wait slots).
                    nc.tensor.matmul(
                        z[:, g, :], b1r[0:1, g * H : (g + 1) * H], ones[:],
                        start=True, stop=False,
                    )
                for g in range(4):
                    nc.tensor.matmul(
                        z[:, g, :], wa[:, g * H : (g + 1) * H], xa,
                        start=False, stop=(t == 0),
                    )
                if t > 0:
                    for g in range(4):
                        nc.tensor.matmul(
                            z[:, g, :], u1[:, g * H : (g + 1) * H], st["h"][:],
                            start=False, stop=True,
                        )
                elementwise(st, z)

            def dec_step(st):
                # z = [pred;1] @ [W2;b2] + h @ U2
                z = st["pool"].tile([H, 4, HALF], FP32, tag=st["tag"], name="z" + st["tag"])
                for g in range(4):
                    nc.tensor.matmul(
                        z[:, g, :], w2[:, g * H : (g + 1) * H], st["pred"][:],
                        start=True, stop=False,
                    )
                for g in range(4):
                    nc.tensor.matmul(
                        z[:, g, :], u2[:, g * H : (g + 1) * H], st["h"][:],
                        start=False, stop=True,
                    )
                elementwise(st, z)

            def head(st, k):
                hd = st["pool"].tile([H, 3, HALF], FP32, tag=st["tag"], name="hd" + st["tag"])
                # 1x1 matmul absorbing the PSUM-slot WAR wait so the x1 matmul
                # carries only its RAW dependency.
                wdm = w1[0:1, 0:1]
                nc.tensor.matmul(
                    hd[0:1, 0, 0:1], wdm, wdm,
                    start=True, stop=True, skip_group_check=True,
                )
                nc.tensor.matmul(hd[:, 0, :], wd1[:], st["h"][:])
                nc.vector.tensor_scalar(
                    st["x1"][:], hd[:, 0, :], bd1[:, 0:1], 0.0, ALU.add, ALU.max
                )
                nc.tensor.matmul(hd[:, 1, :], wd1[:], st["x1"][:])
                nc.vector.tensor_scalar(
                    st["x2"][:], hd[:, 1, :], bd1[:, 0:1], 0.0, ALU.add, ALU.max
                )
                nc.tensor.matmul(hd[:, 2, :], wd[:], st["x2"][:])
                nc.vector.tensor_scalar(
                    st["pred"][0:F, :], hd[0:F, 2, :], bd[:, 0:1], None, ALU.add
                )
                nc.sync.dma_start(
                    out_d[k, :, st["off"] : st["off"] + HALF], st["pred"][0:F, :]
                )

            # ---- warmup scan over the input sequence ----
            for t in range(T):
                for st in halves:
                    warm_step(st, t)

            # ---- autoregressive decode ----
            for st in halves:
                head(st, 0)
            for k in range(1, OUT):
                for st in halves:
                    dec_step(st)
                for st in halves:
                    head(st, k)

    nc.compile()
    return nc


_NC_CACHE = None


def _get_nc():
    global _NC_CACHE
    if _NC_CACHE is None:
        _NC_CACHE = build_nc()
    return _NC_CACHE


def _prep_weights(W1, U1, b1, W2, U2, b2, Wd1, bd1, Wd, bd):
    f16 = np.float16
    perm = np.concatenate(
        [np.arange(0, 128), np.arange(128, 256), np.arange(384, 512), np.arange(256, 384)]
    )
    W1p, U1p, b1p = W1[:, perm], U1[:, perm], b1[perm]
    W2p, U2p, b2p = W2[:, perm], U2[:, perm], b2[perm]
    w1dup = np.ascontiguousarray(np.concatenate([W1p, W1p], axis=0), f16)
    w2aug = np.ascontiguousarray(np.concatenate([W2p, b2p[None, :]], axis=0), f16)
    return {
        "w1dup": w1dup,
        "b1row": np.ascontiguousarray(b1p[None, :], f16),
        "u1": np.ascontiguousarray(U1p, f16),
        "w2aug": w2aug,
        "u2": np.ascontiguousarray(U2p, f16),
        "wd1": np.ascontiguousarray(Wd1, f16),
        "wd": np.ascontiguousarray(
            np.concatenate([Wd, np.zeros((H, H - F), np.float32)], axis=1), f16
        ),
        "bd1": np.ascontiguousarray(bd1[:, None], np.float32),
        "bd": np.ascontiguousarray(bd[:, None], np.float32),
        "onesrow": np.ones((1, HALF), f16),
    }


def _prep_x(inputs):
    # inputs [Bn, T, F] -> [2F=128, T/2, Bn] fp16: even timesteps on rows
    # 0-63, odd timesteps on rows 64-127
    x16 = np.asarray(inputs, np.float16)              # [Bn, T, F]
    Bn = x16.shape[0]
    v = x16.reshape(Bn, TP, 2, F)
    return np.ascontiguousarray(v.transpose(2, 3, 1, 0)).reshape(H, TP, Bn)


def _preprocess(inputs, W1, U1, b1, W2, U2, b2, Wd1, bd1, Wd, bd):
    shared = _prep_weights(W1, U1, b1, W2, U2, b2, Wd1, bd1, Wd, bd)
    xpk = _prep_x(inputs)  # [128, T/2, B]
    in_maps = []
    for i in range(NCORES):
        m = dict(shared)
        m["x"] = xpk[:, :, i * BC : (i + 1) * BC]
        in_maps.append(m)
    return in_maps


def kernel(**inputs):
    global LAST_RESULT
    args = {k: np.asarray(v) for k, v in inputs.items()}
    in_maps = _preprocess(**args)
    nc = _get_nc()
    res = run_bass_kernel_spmd(nc, in_maps, list(range(NCORES)))
    LAST_RESULT = res
    outs = [res.results[i]["out"] for i in range(NCORES)]  # each [OUT, F, BC]
    full = np.concatenate(outs, axis=2)  # [OUT, F, B]
    return np.ascontiguousarray(np.transpose(full, (2, 0, 1)), np.float32)
